# revision 8
# baseline (speedup 1.0000x reference)
"""TRN2 Bass/Tile kernel for nn_BlockSparseMoE (T=2048, D=1024, F=2048, E=8, top-2).

Expert parallelism across the 8 NeuronCores: core c owns expert c, sees the
full token stream, and produces a partial output that the host sums.

Per-core device pipeline (all phases in one NEFF):
  R   router logits via plain-fp32 matmuls (gate_w^T stationary, x^T streamed
      from a host-swizzled q-major layout so each 512-token chunk arrives in
      one DMA), PE-transposed to token-major; top-2-of-8 via DVE max8; combine
      coef by value matching + pairwise-renormalized softmax weights.
  P   matmul-based prefix sum over the selection mask -> compact slot index
      pos[t] for every selected token (rejects get slots >= C).
  S   selection matrix Psel[t, i] = (pos[t] == i), built by one batched DVE
      is_equal per 128-slot tile against broadcast iota rows. Slot->token ids
      and slot coefs come from tiny fp32 matmuls Psel^T @ [iota-T, coef].
  G   indirect row-gather of bf16 tokens, PE-transposed to d-major bf16.
  M12 a^T = W1 x_c^T, b^T = V1 x_c^T (bf16), h^T = silu(a^T) * b^T -> bf16.
  M3  y = h @ W2 token-major (lhsT = h^T slices, bf16), scaled by slot coef,
      then one indirect row-scatter per 128-slot tile into the partial out.

All FFN weights are bf16 (halves HBM traffic; PE rate is 1 cycle/row either
way). Routing stays exact fp32 end-to-end: the input has a token whose
rank-2/3 logit gap is ~5e-6, so reduced-precision logits could flip an
expert choice and blow the error budget. w1/v1 are host-packed in pairs
(one 1MB DMA per two experts-rows) and streamed; w2 is host-packed into
4 tiles and preloaded during the routing/gather phases so M3 never stalls.
Capacity C is static per NEFF; host picks the smallest compiled C that fits
the actual routing counts (cheap argsort on host, used only for shape choice).
"""

import os

import numpy as np

import concourse.bass as bass
import concourse.mybir as mybir
import concourse.tile as tile
from concourse import bacc
from concourse.bass_utils import run_bass_kernel_spmd
from concourse.masks import make_identity, make_upper_triangular

f32 = mybir.dt.float32
bf16 = mybir.dt.bfloat16
i32 = mybir.dt.int32
AF = mybir.ActivationFunctionType
OP = mybir.AluOpType

_PHASES = (set(os.environ["MOE_PHASES"].split(","))
           if os.environ.get("MOE_PHASES") else None)
_REPS = int(os.environ.get("MOE_REPS", "1"))

P = 128
T = 2048
D = 1024
F = 2048
E = 8
NT = T // P   # 16 token tiles
ND = D // P   # 8 d tiles
NF = F // P   # 16 f tiles
NQ = 4        # routing token chunks of 512
NW = NF // 2  # w1+v1 pair tiles
NG2 = NF // 4  # w2 group tiles


def _chunks(C):
    """Slot chunks, each <=512 wide (PSUM bank) and 128-aligned."""
    if C <= 512:
        return [C]
    return [512, C - 512]


def build_moe(C, reps=None):
    global _REPS
    if reps is not None:
        _REPS = reps
    assert C % P == 0 and 512 <= C <= 1024
    NCTOK = C // P
    CHS = _chunks(C)
    CH0 = CHS[0]

    nc = bacc.Bacc("TRN2", target_bir_lowering=False, debug=False)

    xq = nc.dram_tensor("xq", [NQ, P, ND * 512], f32, kind="ExternalInput").ap()
    xh = nc.dram_tensor("xh", [T, D], bf16, kind="ExternalInput").ap()
    gwT = nc.dram_tensor("gwT", [D, E], f32, kind="ExternalInput").ap()
    wv = nc.dram_tensor("wv", [NW, P, 4 * ND * P], bf16, kind="ExternalInput").ap()
    w2g = nc.dram_tensor("w2g", [NG2, P, 4 * D], bf16, kind="ExternalInput").ap()
    outp = nc.dram_tensor("outp", [T, D], f32, kind="ExternalOutput").ap()

    with tile.TileContext(nc) as tc:
        with (
            tc.tile_pool(name="const", bufs=1) as cpool,
            tc.tile_pool(name="route", bufs=1) as rpool,
            tc.tile_pool(name="xqs", bufs=2) as xqpool,
            tc.tile_pool(name="psel", bufs=2) as selpool,
            tc.tile_pool(name="xc", bufs=2) as xcpool,
            tc.tile_pool(name="xct", bufs=2 * ND) as xctpool,
            tc.tile_pool(name="wv", bufs=2) as wvpool,
            tc.tile_pool(name="w2", bufs=NG2) as w2pool,
            tc.tile_pool(name="ht", bufs=2 * NF) as htpool,
            tc.tile_pool(name="ysb", bufs=3) as ypool,
            tc.tile_pool(name="small", bufs=2) as spool,
            tc.tile_pool(name="idxcf", bufs=2 * NCTOK) as icpool,
            tc.tile_pool(name="psum", bufs=1, space="PSUM") as psp,
        ):
            # ---------------- constants ----------------
            ident = cpool.tile([P, P], f32, tag="ident")
            make_identity(nc, ident[:])
            identb = cpool.tile([P, P], bf16, tag="identb")
            make_identity(nc, identb[:])
            ut128 = cpool.tile([P, P], f32, tag="ut128")
            make_upper_triangular(nc, ut128[:], val=1.0, diag=True)
            sut16 = cpool.tile([NT, NT], f32, tag="sut16")
            make_upper_triangular(nc, sut16[:], val=1.0, diag=False)
            ones_col = cpool.tile([P, 1], f32, tag="ones_col")
            nc.vector.memset(ones_col[:], 1.0)
            ones_row = cpool.tile([1, P], f32, tag="ones_row")
            nc.vector.memset(ones_row[:], 1.0)
            # token-id iota (column-major tiles): val[p, j] = C + p + 128*j
            iotaC_f = cpool.tile([P, NT], f32, tag="iotaC_f")
            iotaC_i = cpool.tile([P, NT], i32, tag="iotaC_i")
            nc.gpsimd.iota(iotaC_i[:], pattern=[[P, NT]], base=C,
                           channel_multiplier=1)
            nc.vector.tensor_copy(out=iotaC_f[:], in_=iotaC_i[:])
            # token-id minus T, for slot->token extraction (pad slots -> id T)
            iotaT_f = cpool.tile([P, NT], f32, tag="iotaT_f")
            iotaT_i = cpool.tile([P, NT], i32, tag="iotaT_i")
            nc.gpsimd.iota(iotaT_i[:], pattern=[[P, NT]], base=-T,
                           channel_multiplier=1)
            nc.vector.tensor_copy(out=iotaT_f[:], in_=iotaT_i[:])
            # slot-id row replicated on all partitions: val[p, i] = i
            slotrow_f = cpool.tile([P, C], f32, tag="slotrow_f")
            slotrow_i = cpool.tile([P, C], i32, tag="slotrow_i")
            nc.gpsimd.iota(slotrow_i[:], pattern=[[1, C]], base=0,
                           channel_multiplier=0)
            nc.vector.tensor_copy(out=slotrow_f[:], in_=slotrow_i[:])

            gw_sb = cpool.tile([P, ND, E], f32, tag="gw")
            nc.sync.dma_start(
                out=gw_sb[:], in_=gwT[:, :].rearrange("(dt p) e -> p dt e", p=P)
            )

            def _emit_body():
                # ---------------- phase R: routing ----------------
                xq_t = [None] * NQ
                for q in range(NQ):
                    xq_t[q] = xqpool.tile([P, ND * 512], f32, tag="xq",
                                          name=f"xq_{q}")
                    nc.sync.dma_start(out=xq_t[q][:], in_=xq[q, :, :])
                lg3 = rpool.tile([P, NT, E], f32, tag="lg3")
                mx3 = rpool.tile([P, NT, E], f32, tag="mx3")
                for q in range(NQ):
                    lt_ps = psp.tile([E, 512], f32, tag="rt", bufs=2, name="lt_ps")
                    for d in range(ND):
                        nc.tensor.matmul(
                            out=lt_ps[:], lhsT=gw_sb[:, d, :],
                            rhs=xq_t[q][:, d * 512:(d + 1) * 512],
                            start=(d == 0), stop=(d == ND - 1),
                        )
                    lt_sb = rpool.tile([E, 512], f32, tag="lt", bufs=2,
                                       name=f"lt_{q}")
                    nc.vector.tensor_copy(out=lt_sb[:], in_=lt_ps[:])
                    for jj in range(4):
                        j = 4 * q + jj
                        tp_ps = psp.tile([P, E], f32, tag="rt", bufs=2,
                                         name="tp_ps")
                        nc.tensor.transpose(
                            out=tp_ps[:], in_=lt_sb[:, jj * P:(jj + 1) * P],
                            identity=ident[0:E, 0:E],
                        )
                        lg_j = lg3[:, j, :]
                        nc.vector.tensor_copy(out=lg_j, in_=tp_ps[:])
                        nc.vector.max(out=mx3[:, j, :], in_=lg_j)

                m1 = mx3[:, :, 0]
                m2 = mx3[:, :, 1]
                l0 = lg3[:, :, 0]
                dm = rpool.tile([P, NT], f32, tag="dm")
                nc.vector.tensor_sub(out=dm[:], in0=m2, in1=m1)
                ex = rpool.tile([P, NT], f32, tag="ex")
                nc.scalar.activation(ex[:], dm[:], AF.Exp)
                w1c = rpool.tile([P, NT], f32, tag="w1c")
                nc.vector.tensor_scalar(w1c[:], ex[:], 1.0, scalar2=None, op0=OP.add)
                nc.vector.reciprocal(out=w1c[:], in_=w1c[:])
                w2c = rpool.tile([P, NT], f32, tag="w2c")
                nc.vector.tensor_sub(
                    out=w2c[:], in0=ones_col[:].to_broadcast([P, NT]), in1=w1c[:]
                )
                eq1 = rpool.tile([P, NT], f32, tag="eq1")
                nc.vector.tensor_tensor(out=eq1[:], in0=l0, in1=m1, op=OP.is_equal)
                eq2 = rpool.tile([P, NT], f32, tag="eq2")
                nc.vector.tensor_tensor(out=eq2[:], in0=l0, in1=m2, op=OP.is_equal)
                coefa = rpool.tile([P, NT], f32, tag="coefa")
                nc.vector.tensor_mul(out=coefa[:], in0=eq1[:], in1=w1c[:])
                coefb = rpool.tile([P, NT], f32, tag="coefb")
                nc.vector.tensor_mul(out=coefb[:], in0=eq2[:], in1=w2c[:])
                nc.vector.tensor_add(out=coefa[:], in0=coefa[:], in1=coefb[:])
                mask = rpool.tile([P, NT], f32, tag="mask")
                nc.vector.tensor_add(out=mask[:], in0=eq1[:], in1=eq2[:])

                if _PHASES and "P" not in _PHASES:
                    return
                # ---------------- phase P: prefix-sum compaction ----------------
                ps_ps = psp.tile([P, NT], f32, tag="rt", bufs=2, name="ps_ps")
                nc.tensor.matmul(
                    out=ps_ps[:], lhsT=ut128[:], rhs=mask[:], start=True, stop=False
                )
                cs_ps = psp.tile([NT, 1], f32, tag="rt", bufs=2, name="cs_ps")
                nc.tensor.matmul(
                    out=cs_ps[:], lhsT=mask[:], rhs=ones_col[:], start=True, stop=True
                )
                cs_sb = spool.tile([NT, 1], f32, tag="cs_sb")
                nc.vector.tensor_copy(out=cs_sb[:], in_=cs_ps[:])
                or_ps = psp.tile([1, NT], f32, tag="rt", bufs=2, name="or_ps")
                nc.tensor.matmul(
                    out=or_ps[:], lhsT=cs_sb[:], rhs=sut16[:], start=True, stop=True
                )
                or_sb = spool.tile([1, NT], f32, tag="or_sb")
                nc.vector.tensor_copy(out=or_sb[:], in_=or_ps[:])
                nc.tensor.matmul(
                    out=ps_ps[:], lhsT=ones_row[:], rhs=or_sb[:], start=False,
                    stop=True,
                )
                # selected: pos = S - 1 ; rejected: pos = C + t - S  (>= C)
                posa = rpool.tile([P, NT], f32, tag="posa")
                nc.vector.tensor_scalar(
                    posa[:], ps_ps[:], 1.0, scalar2=None, op0=OP.subtract
                )
                posf = rpool.tile([P, NT], f32, tag="posf")
                nc.vector.tensor_sub(out=posf[:], in0=iotaC_f[:], in1=ps_ps[:])
                mask_i = rpool.tile([P, NT], i32, tag="mask_i")
                nc.vector.tensor_copy(out=mask_i[:], in_=mask[:])
                nc.vector.copy_predicated(out=posf[:], mask=mask_i[:], data=posa[:])

                if _PHASES and "G" not in _PHASES:
                    return
                # ------- phase S: slot->token ids + slot coefs via Psel matmuls ----
                idx_sb = [None] * NCTOK
                cf_sb = [None] * NCTOK
                rhs2 = rpool.tile([P, NT, 2], f32, tag="rhs2")
                nc.vector.tensor_copy(out=rhs2[:, :, 0], in_=iotaT_f[:])
                nc.vector.tensor_copy(out=rhs2[:, :, 1], in_=coefa[:])
                for i in range(NCTOK):
                    psel = selpool.tile([P, NT, P], f32, tag="psel",
                                        name=f"psel_{i}")
                    nc.vector.tensor_tensor(
                        out=psel[:],
                        in0=posf[:, :, None].to_broadcast([P, NT, P]),
                        in1=slotrow_f[:, None, i * P:(i + 1) * P]
                            .to_broadcast([P, NT, P]),
                        op=OP.is_equal,
                    )
                    idcf_ps = psp.tile([P, 2], f32, tag="ic", bufs=2,
                                       name=f"idcf_ps_{i}")
                    for j in range(NT):
                        nc.tensor.matmul(
                            out=idcf_ps[:],
                            lhsT=psel[:, j, :],
                            rhs=rhs2[:, j, :],
                            start=(j == 0),
                            stop=(j == NT - 1),
                        )
                    idf = icpool.tile([P, 1], f32, tag="idf", name=f"idf_{i}")
                    # token id; pad slots land at T (out of bounds, skipped later)
                    nc.vector.tensor_scalar(
                        idf[:], idcf_ps[:, 0:1], float(T), scalar2=None, op0=OP.add
                    )
                    idx_sb[i] = icpool.tile([P, 1], i32, tag="idx", name=f"idx_{i}")
                    nc.vector.tensor_copy(out=idx_sb[i][:], in_=idf[:])
                    cf_sb[i] = icpool.tile([P, 1], f32, tag="cf", name=f"cf_{i}")
                    nc.vector.tensor_copy(out=cf_sb[i][:], in_=idcf_ps[:, 1:2])

                # w2 preload: emitted here so the DMAs land well before M3
                w2_sb = [None] * NG2
                for g in range(NG2):
                    w2_sb[g] = w2pool.tile([P, 4 * D], bf16, tag="w2",
                                           name=f"w2_{g}")
                    nc.sync.dma_start(out=w2_sb[g][:], in_=w2g[g, :, :])

                # ------- phase G: gather compact bf16 tokens + PE-transpose ------
                def chof(i):
                    glo = i * P
                    ch = 0 if glo < CH0 else 1
                    return ch, glo - ch * CH0

                xcT = [[None] * len(CHS) for _ in range(ND)]
                for d in range(ND):
                    for ch in range(len(CHS)):
                        xcT[d][ch] = xctpool.tile([P, CHS[ch]], bf16,
                                                  tag=f"xct{ch}", bufs=ND,
                                                  name=f"xct_{d}_{ch}")
                for i in range(NCTOK):
                    ch, loc = chof(i)
                    xc_t = xcpool.tile([P, D], bf16, tag="xc", name=f"xc_{i}")
                    nc.gpsimd.indirect_dma_start(
                        out=xc_t[:],
                        out_offset=None,
                        in_=(xh[:, :] if os.environ.get("MOE_SIM_SAFE")
                             else xh[0:P, :]),
                        in_offset=bass.IndirectOffsetOnAxis(
                            ap=idx_sb[i][:, 0:1], axis=0
                        ),
                        bounds_check=T - 1,
                        oob_is_err=False,
                    )
                    for d in range(ND):
                        tr_ps = psp.tile([P, P], bf16, tag="rt", bufs=2,
                                         name="tr_ps")
                        nc.tensor.transpose(
                            out=tr_ps[:],
                            in_=xc_t[:, d * P:(d + 1) * P],
                            identity=identb[:],
                        )
                        nc.vector.tensor_copy(
                            out=xcT[d][ch][:, loc:loc + P], in_=tr_ps[:]
                        )

                if _PHASES and "M12" not in _PHASES:
                    return
                # ---------------- phase M12: h^T = silu(a^T) * b^T ----------------
                hT = [[None] * len(CHS) for _ in range(NF)]
                for f in range(NF):
                    for ch in range(len(CHS)):
                        hT[f][ch] = htpool.tile([P, CHS[ch]], bf16,
                                                tag=f"ht{ch}", bufs=NF,
                                                name=f"ht_{f}_{ch}")
                for c in range(NW):
                    wv_sb = wvpool.tile([P, 4 * ND * P], bf16, tag="wv",
                                        name=f"wv_{c}")
                    nc.sync.dma_start(out=wv_sb[:], in_=wv[c, :, :])
                    for k in range(2):
                        f = 2 * c + k
                        w1o = k * 2 * ND * P
                        v1o = w1o + ND * P
                        for ch in range(len(CHS)):
                            wd = CHS[ch]
                            a_ps = psp.tile([P, CH0], f32, tag="mm", bufs=4,
                                            name="a_ps")
                            b_ps = psp.tile([P, CH0], f32, tag="mm", bufs=4,
                                            name="b_ps")
                            for d in range(ND):
                                nc.tensor.matmul(
                                    out=a_ps[:, :wd],
                                    lhsT=wv_sb[:, w1o + d * P:w1o + (d + 1) * P],
                                    rhs=xcT[d][ch][:],
                                    start=(d == 0), stop=(d == ND - 1),
                                )
                            for d in range(ND):
                                nc.tensor.matmul(
                                    out=b_ps[:, :wd],
                                    lhsT=wv_sb[:, v1o + d * P:v1o + (d + 1) * P],
                                    rhs=xcT[d][ch][:],
                                    start=(d == 0), stop=(d == ND - 1),
                                )
                            s_sb = spool.tile([P, CH0], f32, tag="s_sb")
                            nc.scalar.activation(s_sb[:, :wd], a_ps[:, :wd],
                                                 AF.Sigmoid)
                            nc.vector.tensor_tensor(
                                out=s_sb[:, :wd], in0=s_sb[:, :wd],
                                in1=a_ps[:, :wd], op=OP.mult,
                            )
                            nc.vector.tensor_tensor(
                                out=hT[f][ch][:], in0=s_sb[:, :wd],
                                in1=b_ps[:, :wd], op=OP.mult,
                            )

                if _PHASES and "M3" not in _PHASES:
                    return
                # ---------------- phase M3: y = h @ W2, scale, scatter ------------
                for i in range(NCTOK):
                    ch, lo = chof(i)
                    y_sb = ypool.tile([P, D], f32, tag="y_sb", name=f"y_{i}")
                    for dch in range(2):
                        y_ps = psp.tile([P, 512], f32, tag="mm", bufs=4,
                                        name="y_ps")
                        for g in range(NG2):
                            for j in range(4):
                                f = 4 * g + j
                                nc.tensor.matmul(
                                    out=y_ps[:],
                                    lhsT=hT[f][ch][:, lo:lo + P],
                                    rhs=w2_sb[g][:, j * D + dch * 512:
                                                 j * D + dch * 512 + 512],
                                    start=(f == 0), stop=(f == NF - 1),
                                )
                        nc.scalar.activation(
                            y_sb[:, dch * 512:(dch + 1) * 512], y_ps[:],
                            AF.Copy, scale=cf_sb[i][:, 0:1],
                        )
                    nc.gpsimd.indirect_dma_start(
                        out=(outp[:, :] if os.environ.get("MOE_SIM_SAFE")
                             else outp[0:P, :]),
                        out_offset=bass.IndirectOffsetOnAxis(
                            ap=idx_sb[i][:, 0:1], axis=0
                        ),
                        in_=y_sb[:],
                        in_offset=None,
                        bounds_check=T - 1,
                        oob_is_err=False,
                    )

            for _rep in range(_REPS):
                _emit_body()

    return nc


_NC_CACHE = {}


def _get_nc(C, reps=None):
    key = (C, reps if reps is not None else _REPS)
    if key not in _NC_CACHE:
        nc = build_moe(C, reps=reps)
        nc.compile()
        _NC_CACHE[key] = nc
    return _NC_CACHE[key]


def _routing_counts(x, gate_w):
    logits = x.astype(np.float32) @ gate_w.astype(np.float32).T
    order = np.argsort(-logits, axis=1)[:, :2]
    return np.bincount(order.ravel(), minlength=E)


_BF16 = mybir.dt.np(mybir.dt.bfloat16)


def _swizzle_w1(w):
    """(F, D) -> [NF, 128, ND*128] with [f, p, dt*128+fc] = w[f*128+fc, dt*128+p]."""
    v = w.reshape(NF, P, ND, P)  # [f, fc, dt, p]
    return np.ascontiguousarray(v.transpose(0, 3, 2, 1).reshape(NF, P, ND * P))

def _pack_wv(w1e, v1e):
    """Pair tiles: [NW, P, 4096] = [w1_2c | v1_2c | w1_2c+1 | v1_2c+1]."""
    s1 = _swizzle_w1(w1e)
    sv = _swizzle_w1(v1e)
    out = np.empty((NW, P, 4 * ND * P), dtype=np.float32)
    out[:, :, 0 * 1024:1 * 1024] = s1[0::2]
    out[:, :, 1 * 1024:2 * 1024] = sv[0::2]
    out[:, :, 2 * 1024:3 * 1024] = s1[1::2]
    out[:, :, 3 * 1024:4 * 1024] = sv[1::2]
    return out.astype(_BF16)


def _pack_w2(w2e):
    """(F, D) -> [NG2, P, 4*D] with [g, p, j*D+d] = w2[(4g+j)*128+p, d]."""
    v = w2e.reshape(NG2, 4, P, D).transpose(0, 2, 1, 3).reshape(NG2, P, 4 * D)
    return np.ascontiguousarray(v).astype(_BF16)


def _pack_xq(x):
    """(T, D) -> [NQ, P, ND*512] with [q, p, dt*512+t] = x[q*512+t, dt*128+p]."""
    v = x.reshape(NQ, 512, ND, P).transpose(0, 3, 2, 1).reshape(NQ, P, ND * 512)
    return np.ascontiguousarray(v)


def make_in_maps(x, gate_w, w1, v1, w2):
    x = np.ascontiguousarray(x, dtype=np.float32)
    gate_w = np.ascontiguousarray(gate_w, dtype=np.float32)
    xqh = _pack_xq(x)
    xhh = x.astype(_BF16)
    in_maps = []
    for c in range(E):
        perm = np.concatenate(([c], np.delete(np.arange(E), c)))
        in_maps.append({
            "xq": xqh,
            "xh": xhh,
            "gwT": np.ascontiguousarray(gate_w[perm].T),
            "wv": _pack_wv(w1[c], v1[c]),
            "w2g": _pack_w2(w2[c]),
        })
    return in_maps


def kernel(x, gate_w, w1, v1, w2):
    x = np.ascontiguousarray(x, dtype=np.float32)
    gate_w = np.ascontiguousarray(gate_w, dtype=np.float32)
    w1 = np.ascontiguousarray(w1, dtype=np.float32)
    v1 = np.ascontiguousarray(v1, dtype=np.float32)
    w2 = np.ascontiguousarray(w2, dtype=np.float32)

    counts = _routing_counts(x, gate_w)
    C = max(640, P * int(np.ceil(counts.max() / P)))
    nc = _get_nc(C)

    in_maps = make_in_maps(x, gate_w, w1, v1, w2)
    res = run_bass_kernel_spmd(nc, in_maps, core_ids=list(range(E)))
    out = np.zeros((T, D), dtype=np.float32)
    for r in res.results:
        out += r["outp"]
    return out
